# revision 16
# baseline (speedup 1.0000x reference)
"""Attentional Factorization Machine kernel for 8 Trainium2 NeuronCores.

Data-parallel over batch: 1024 rows -> 128 per core. Per core the field-pair
products hp are built by cyclic-delta enumeration (fp16, DVE 2x) with the
deltas of one parity merged into a single strided instruction (broadcast
stride-0 first operand) per 4-row chunk, and the last three deltas built by
the otherwise-idle GPSIMD engine; the attention MLP mm1 runs on the PE with
W stationary as a single 780-column matmul per row (the fp32 output spans
two PSUM banks); the relu+bias eviction of aw (PSUM->SBUF, contiguous 780)
is split between the scalar engine (activation, bias fused) and DVE (two-op
tensor_scalar); the per-pair scores and p_w projections accumulate in fp32
via one-hot stationary matmuls (one 780-column matmul per row per target)
packed across PE column groups (rows are processed in quads, one row per
column group, so quartets occupy all 4 groups concurrently). Softmax +
combine happen on-chip in a [128, 780] layout; exp is applied without max
subtraction (logits are bounded, softmax is shift-invariant).
"""
import sys
for _p in ("/opt/trn_rl_repo",):
    if _p not in sys.path:
        sys.path.insert(0, _p)

import numpy as np

import concourse.bass as bass
import concourse.bacc as bacc
import concourse.mybir as mybir
import concourse.tile as tile
from concourse.ap import AP

F32 = mybir.dt.float32
F16 = mybir.dt.float16
AF = mybir.ActivationFunctionType
ALU = mybir.AluOpType
AXIS = mybir.AxisListType

FLD = 40
NDELTA = 20
P = 780
HALF = 390
ROWQ = 800  # per-row hp stride (20 deltas x 40, incl. junk pad for d=20)

_BLOCKS = (4, 8, 12, 16, 16, 16, 16, 16, 16, 8)
_GP_DELTAS = (18, 19, 20)  # deltas built by GPSIMD (all blocks but first)
_HP_CHUNK = 4  # rows per DVE hp-build instruction pair


def _win(base, dims):
    """Raw AP from base slice with explicit [stride, size] free dims."""
    pdim = list(base.ap[0])
    return AP(base.tensor, base.offset, [pdim] + [list(d) for d in dims])


def build(nc, B_c=128, blocks=_BLOCKS, gp_deltas=_GP_DELTAS, dve_extra_mod=8):
    assert B_c == 128 and sum(blocks) == 128
    assert all(nb % 4 == 0 for nb in blocks)

    xTa_d = nc.dram_tensor("xTa", [128, B_c, 60], F16, kind="ExternalInput").ap()
    xTb_d = nc.dram_tensor("xTb", [128, B_c, 60], F16, kind="ExternalInput").ap()
    wT_d = nc.dram_tensor("wT", [128, 128], F16, kind="ExternalInput").ap()
    bias_d = nc.dram_tensor("bias", [128, 1], F32, kind="ExternalInput").ap()
    Zh_d = nc.dram_tensor("Zh", [128, 64], F16, kind="ExternalInput").ap()
    Zg_d = nc.dram_tensor("Zg", [128, 64], F16, kind="ExternalInput").ap()
    pb_d = nc.dram_tensor("pb", [128, 1], F32, kind="ExternalInput").ap()
    out_d = nc.dram_tensor("out", [B_c, 1], F32, kind="ExternalOutput").ap()

    NBMAX = max(blocks)
    all_odds = list(range(1, NDELTA + 1, 2))
    all_evens = list(range(2, NDELTA + 1, 2))
    dve_odds = [d for d in all_odds if d not in gp_deltas]
    dve_evens = [d for d in all_evens if d not in gp_deltas]
    for grp in (dve_odds, dve_evens):
        assert all(b - a == 2 for a, b in zip(grp, grp[1:])), grp

    with tile.TileContext(nc) as tc:
        with (
            tc.tile_pool(name="const", bufs=1) as cpool,
            tc.tile_pool(name="hp", bufs=3) as hpool,
            tc.tile_pool(name="relu", bufs=24) as rpool,
            tc.tile_pool(name="awps", bufs=2, space="PSUM") as awpool,
            tc.tile_pool(name="accps", bufs=1, space="PSUM") as accpool,
        ):
            wT_s = cpool.tile([128, 128], F16, tag="wT")
            bias_s = cpool.tile([128, 1], F32, tag="bias")
            Zh_s = cpool.tile([128, 64], F16, tag="Zh")
            Zg_s = cpool.tile([128, 64], F16, tag="Zg")
            pb_s = cpool.tile([128, 1], F32, tag="pb")
            xTa = cpool.tile([128, B_c, 60], F16, tag="xTa")
            xTb = cpool.tile([128, B_c, 60], F16, tag="xTb")
            warm0 = cpool.tile([128, 1], F32, tag="warm0")
            warm1 = cpool.tile([128, 1], F32, tag="warm1")
            warm2 = cpool.tile([128, 2], F16, tag="warm2")

            # Warmups with no data deps: load the exp table set on ACT
            # (contains relu too, so exactly one ACT_TABLE_LOAD, paid at t~0)
            # and pay any first-use cost of the gpsimd tensor_tensor path.
            nc.vector.memset(warm0[:], 0.0)
            nc.scalar.activation(warm1[:], warm0[:], AF.Exp)
            if gp_deltas:
                nc.gpsimd.memset(warm2[:], 0.0)
                nc.gpsimd.tensor_mul(warm2[:, 0:1], warm2[:, 1:2],
                                     warm2[:, 1:2])

            # DMA order: first block's x slices first (they gate the first
            # hp build), then the consts needed earliest, then the rest.
            nb0 = blocks[0]
            nc.sync.dma_start(xTa[:, 0:nb0, :], xTa_d[:, 0:nb0, :])
            nc.sync.dma_start(xTb[:, 0:nb0, :], xTb_d[:, 0:nb0, :])
            nc.sync.dma_start(wT_s[:], wT_d[:])
            nc.sync.dma_start(bias_s[:], bias_d[:])
            nc.sync.dma_start(Zh_s[:], Zh_d[:])
            nc.sync.dma_start(Zg_s[:], Zg_d[:])
            nc.sync.dma_start(pb_s[:], pb_d[:])

            sc_h0 = accpool.tile([128, 512], F32, tag="sc_h0")
            sc_h1 = accpool.tile([128, 512], F32, tag="sc_h1")
            g_h0 = accpool.tile([128, 512], F32, tag="g_h0")
            g_h1 = accpool.tile([128, 512], F32, tag="g_h1")
            sc_h = [sc_h0, sc_h1]
            g_h = [g_h0, g_h1]

            scg_q = []  # pending quads: (hp3, t, rows) with rows=[(k,j,relu)]

            def emit_mm1(hp3, k, first):
                aw = awpool.tile([128, 1024], F32, tag="aw")
                for h in (0, 1):
                    bi = nc.tensor.matmul(
                        aw[:, 512 * h:512 * h + HALF],
                        wT_s[:],
                        hp3[:, k, h * HALF:(h + 1) * HALF],
                        start=True, stop=True,
                    )
                    if not (first and h == 0):
                        bi.ins.ldweights = False
                return aw

            def emit_evict(aw, on_dve):
                relu = rpool.tile([128, P], F16, tag="relu")
                aw_v = aw[:].rearrange("a (u q) -> a u q", q=512)[:, :, 0:HALF]
                relu_v = relu[:].rearrange("a (u q) -> a u q", q=HALF)
                if on_dve:
                    nc.vector.tensor_scalar(
                        relu_v, aw_v, bias_s[:], 0.0, op0=ALU.add, op1=ALU.max)
                else:
                    nc.scalar.activation(relu_v, aw_v, AF.Relu, bias=bias_s[:])
                return relu

            def emit_scg(rec, half):
                hp3o, t, rows = rec
                st, sp = (t == 0), (t == 31)
                for qi in ((0, 1) if half == 0 else (2, 3)):
                    ra = rows[qi % 4]
                    rb = rows[(qi + 1) % 4]
                    rc = rows[(qi + 2) % 4]
                    rd = rows[(qi + 3) % 4]
                    for (k, j, relu), h, is_sc in (
                        (ra, 0, True), (rb, 1, True),
                        (rc, 0, False), (rd, 1, False),
                    ):
                        if is_sc:
                            dst, Z = sc_h[h], Zh_s
                            mov = relu[:, h * HALF:(h + 1) * HALF]
                        else:
                            dst, Z = g_h[h], Zg_s
                            mov = hp3o[:, k, h * HALF:(h + 1) * HALF]
                        nc.tensor.matmul(
                            dst[32 * j:32 * j + 32, 0:HALF],
                            Z[:, 32 - t:64 - t],
                            mov,
                            start=st, stop=sp,
                            tile_position=(0, 32 * j),
                            skip_group_check=True,
                        )

            def emit_hp_dve(hp3, bs, r0, nr, deltas):
                """One merged multiply over `deltas` (same parity, stride 2)
                for block-local rows r0:r0+nr.

                pair columns: delta d block at (d-1)*40, width 40 (d=20
                writes 40 with the last 20 junk -> cols 780:800 pad).
                in0 = x[0:40] broadcast over deltas; in1 = shifted window.
                """
                nd = len(deltas)
                d0 = deltas[0]
                col0 = (d0 - 1) * FLD
                ob = hp3[:, r0:r0 + nr, col0:col0 + 1]
                out_ap = _win(ob, [[ROWQ, nr], [2 * FLD, nd], [1, FLD]])
                i0b = xTa[:, bs + r0:bs + r0 + nr, 0:1]
                in0 = _win(i0b, [[60, nr], [0, nd], [1, FLD]])
                if d0 % 2 == 0:
                    i1b = xTa[:, bs + r0:bs + r0 + nr, d0:d0 + 1]
                else:
                    i1b = xTb[:, bs + r0:bs + r0 + nr, d0 - 1:d0]
                in1 = _win(i1b, [[60, nr], [2, nd], [1, FLD]])
                nc.vector.tensor_mul(out_ap, in0, in1)

            def emit_hp_gp(hp3, bs, NB, d):
                col0 = (d - 1) * FLD
                out_ap = _win(hp3[:, 0:NB, col0:col0 + 1],
                              [[ROWQ, NB], [1, FLD]])
                in0 = _win(xTa[:, bs:bs + NB, 0:1], [[60, NB], [1, FLD]])
                if d % 2 == 0:
                    i1b = xTa[:, bs:bs + NB, d:d + 1]
                else:
                    i1b = xTb[:, bs:bs + NB, d - 1:d]
                in1 = _win(i1b, [[60, NB], [1, FLD]])
                nc.gpsimd.tensor_mul(out_ap, in0, in1)

            bs = 0
            tq = 0  # global quad index == one-hot position mp
            for bi_, NB in enumerate(blocks):
                if bi_ > 0:
                    nc.sync.dma_start(xTa[:, bs:bs + NB, :],
                                      xTa_d[:, bs:bs + NB, :])
                    nc.sync.dma_start(xTb[:, bs:bs + NB, :],
                                      xTb_d[:, bs:bs + NB, :])

                hp = hpool.tile([128, NBMAX * ROWQ], F16, tag="hp")
                hp3 = hp[:].rearrange("e (b q) -> e b q", q=ROWQ)

                use_gp = bi_ > 0 and len(gp_deltas) > 0
                if use_gp:
                    for d in gp_deltas:
                        emit_hp_gp(hp3, bs, NB, d)
                    odds, evens = dve_odds, dve_evens
                else:
                    odds, evens = all_odds, all_evens

                for q0 in range(0, NB, 4):
                    # hp for this quad's rows (both parities)
                    for rc in range(q0, q0 + 4, _HP_CHUNK):
                        emit_hp_dve(hp3, bs, rc, _HP_CHUNK, odds)
                        emit_hp_dve(hp3, bs, rc, _HP_CHUNK, evens)

                    t = tq
                    tq += 1
                    # eviction split: DVE takes row 1 of each quad, plus row
                    # 3 of every dve_extra_mod-th quad; rest on ACT
                    dve_rows = [1]
                    if dve_extra_mod and t % dve_extra_mod == 3:
                        dve_rows.append(3)
                    rows = []
                    # mm1 + evict for all 4 rows; the scheduler backfills PE
                    # stalls (aw buf reuse waits on evicts) with ready scg
                    for i in range(4):
                        k = q0 + i
                        aw = emit_mm1(hp3, k, first=(i == 0))
                        relu = emit_evict(aw, on_dve=(i in dve_rows))
                        rows.append((k, (bs + k) % 4, relu))
                    # scg runs several quads behind its mm1/evict so all four
                    # relu tiles are long since ready when the quartets issue
                    # -- a deep always-ready pool of 4-wide col-group work
                    if len(scg_q) >= 6:
                        rec = scg_q.pop(0)
                        emit_scg(rec, 0)
                        emit_scg(rec, 1)
                    scg_q.append((hp3, t, rows))
                bs += NB

            while scg_q:
                rec = scg_q.pop(0)
                emit_scg(rec, 0)
                emit_scg(rec, 1)

            # ---- softmax tail ----
            # logits are bounded (|sc| <~ 45) so exp without max subtraction
            # is safe in fp32 and softmax is exactly shift-invariant.
            exp_s = cpool.tile([128, P], F32, tag="exp_s")
            junk = cpool.tile([128, P], F32, tag="junk")
            denom = cpool.tile([128, 1], F32, tag="denom")
            rden = cpool.tile([128, 1], F32, tag="rden")
            numer = cpool.tile([128, 1], F32, tag="numer")
            outc = cpool.tile([128, 1], F32, tag="outc")
            den2 = cpool.tile([128, 2], F32, tag="den2")
            num2 = cpool.tile([128, 2], F32, tag="num2")

            for h in (0, 1):
                nc.scalar.activation(exp_s[:, h * HALF:(h + 1) * HALF],
                                     sc_h[h][:, 0:HALF], AF.Exp,
                                     accum_out=den2[:, h:h + 1])
                nc.vector.scalar_tensor_tensor(
                    junk[:, h * HALF:(h + 1) * HALF],
                    exp_s[:, h * HALF:(h + 1) * HALF],
                    1.0,
                    g_h[h][:, 0:HALF],
                    op0=ALU.mult, op1=ALU.mult,
                    accum_out=num2[:, h:h + 1])
            nc.vector.tensor_reduce(numer[:], num2[:], axis=AXIS.X, op=ALU.add)
            nc.vector.tensor_reduce(denom[:], den2[:], axis=AXIS.X, op=ALU.add)
            nc.vector.reciprocal(rden[:], denom[:])
            nc.vector.tensor_mul(outc[:], numer[:], rden[:])
            nc.vector.tensor_scalar_add(outc[:], outc[:], pb_s[:])
            nc.sync.dma_start(out_d[:], outc[:])

    nc.compile()
    return nc


def make_nc(B_c=128, blocks=_BLOCKS, gp_deltas=_GP_DELTAS, dve_extra_mod=8):
    nc = bacc.Bacc("TRN2", target_bir_lowering=False, debug=False)
    build(nc, B_c=B_c, blocks=blocks, gp_deltas=gp_deltas,
          dve_extra_mod=dve_extra_mod)
    return nc


def perm_for(B_c=128, blocks=None):
    """perm[slot] = global b stored at SBUF slot.

    Slot k belongs to quad k//4 (the one-hot position) and column group
    k%4, so it accumulates into output partition 32*(k%4) + k//4.
    """
    k = np.arange(B_c)
    return 32 * (k % 4) + k // 4


def host_prep_consts(attn_w_w, attn_w_b, attn_h_w, attn_h_b, attn_p_w, attn_p_b):
    wT = np.ascontiguousarray(attn_w_w.T).astype(np.float16)
    bias = attn_w_b.reshape(128, 1).astype(np.float32)
    Zh = np.zeros((128, 64), np.float16)
    Zh[:, 32] = attn_h_w[0].astype(np.float16)
    Zg = np.zeros((128, 64), np.float16)
    Zg[:, 32] = attn_p_w[0].astype(np.float16)
    pb = np.full((128, 1), np.float32(attn_p_b[0]), np.float32)
    return {"wT": wT, "bias": bias, "Zh": Zh, "Zg": Zg, "pb": pb}


def host_prep_x(x_slice, blocks=None):
    # [B_c, F, E] -> two pre-shifted fp16 copies [E, B_c(perm), 60]
    xT = x_slice.transpose(2, 0, 1).astype(np.float16)
    xT = xT[:, perm_for(x_slice.shape[0]), :]
    B_c = x_slice.shape[0]
    xa = np.zeros((128, B_c, 60), np.float16)
    xa[:, :, 0:40] = xT
    xa[:, :, 40:60] = xT[:, :, 0:20]
    xb = np.zeros((128, B_c, 60), np.float16)
    xb[:, :, 0:59] = xa[:, :, 1:60]
    return np.ascontiguousarray(xa), np.ascontiguousarray(xb)


_NC_CACHE = {}


def _get_nc():
    key = (_BLOCKS, _GP_DELTAS)
    if key not in _NC_CACHE:
        _NC_CACHE[key] = make_nc(B_c=128)
    return _NC_CACHE[key]


def kernel(x, attn_w_w, attn_w_b, attn_h_w, attn_h_b, attn_p_w, attn_p_b,
           _trace=False):
    from concourse.bass_utils import run_bass_kernel_spmd
    x = np.asarray(x, np.float32)
    consts = host_prep_consts(np.asarray(attn_w_w), np.asarray(attn_w_b),
                              np.asarray(attn_h_w), np.asarray(attn_h_b),
                              np.asarray(attn_p_w), np.asarray(attn_p_b))
    in_maps = []
    for c in range(8):
        m = dict(consts)
        m["xTa"], m["xTb"] = host_prep_x(x[128 * c:128 * (c + 1)],
                                         blocks=_BLOCKS)
        in_maps.append(m)
    nc = _get_nc()
    res = run_bass_kernel_spmd(nc, in_maps, list(range(8)), trace=_trace)
    out = np.concatenate([res.results[c]["out"][:, 0] for c in range(8)])
    if _trace:
        return out.astype(np.float32), res
    return out.astype(np.float32)


# revision 17
# speedup vs baseline: 1.1387x; 1.1387x over previous
"""Attentional Factorization Machine kernel for 8 Trainium2 NeuronCores.

Data-parallel over batch: 1024 rows -> 128 per core. The 780 field-pair
products per row are processed in TWO PASSES over pair columns (deltas 1-10
= pairs 0:400, deltas 11-20 = pairs 400:780). Per pass: hp products are
built by cyclic-delta enumeration (fp16, DVE 2x, merged strided instruction
per parity with a stride-0 broadcast first operand); the attention MLP mm1
runs on the PE with W stationary (one <=400-column matmul per row, fp32 into
one PSUM bank of a 3-row mega-tile); relu+bias eviction runs on 3-row
batches (PSUM->SBUF) split between the scalar engine and DVE; per-pair
scores and p_w projections accumulate in fp32 via one-hot stationary matmuls
packed across PE column groups (rows processed in quads, one row per column
group). The pass-0 accumulators are copied to SBUF between passes so each
pass needs only 2 persistent PSUM banks, leaving 6 banks for aw tiles.
Softmax + combine happen on-chip in a [128, 780] layout; exp is applied
without max subtraction (logits are bounded, softmax is shift-invariant).
"""
import sys
for _p in ("/opt/trn_rl_repo",):
    if _p not in sys.path:
        sys.path.insert(0, _p)

import numpy as np

import concourse.bass as bass
import concourse.bacc as bacc
import concourse.mybir as mybir
import concourse.tile as tile
from concourse.ap import AP

F32 = mybir.dt.float32
F16 = mybir.dt.float16
AF = mybir.ActivationFunctionType
ALU = mybir.AluOpType
AXIS = mybir.AxisListType

FLD = 40
NDELTA = 20
P = 780
H0 = 400   # pass-0 pair columns (deltas 1..10)
H1 = 380   # pass-1 real pair columns (deltas 11..20, d=20 has 20 junk)
ARQ = 400  # hp arena per-row stride (one pass half, incl. junk pad)

_DMA_CHUNKS = (8, 8, 16, 16, 16, 16, 16, 16, 16)  # row chunks, 8-aligned


def _win(base, dims):
    """Raw AP from base slice with explicit [stride, size] free dims."""
    pdim = list(base.ap[0])
    return AP(base.tensor, base.offset, [pdim] + [list(d) for d in dims])


def build(nc, B_c=128, dve_mod=5):
    assert B_c == 128

    xTa_d = nc.dram_tensor("xTa", [128, B_c, 60], F16, kind="ExternalInput").ap()
    xTb_d = nc.dram_tensor("xTb", [128, B_c, 60], F16, kind="ExternalInput").ap()
    wT_d = nc.dram_tensor("wT", [128, 128], F16, kind="ExternalInput").ap()
    bias_d = nc.dram_tensor("bias", [128, 1], F32, kind="ExternalInput").ap()
    Zh_d = nc.dram_tensor("Zh", [128, 64], F16, kind="ExternalInput").ap()
    Zg_d = nc.dram_tensor("Zg", [128, 64], F16, kind="ExternalInput").ap()
    pb_d = nc.dram_tensor("pb", [128, 1], F32, kind="ExternalInput").ap()
    out_d = nc.dram_tensor("out", [B_c, 1], F32, kind="ExternalOutput").ap()

    with tile.TileContext(nc) as tc:
        with (
            tc.tile_pool(name="const", bufs=1) as cpool,
            tc.tile_pool(name="relu", bufs=10) as rpool,
            tc.tile_pool(name="awps", bufs=2, space="PSUM") as awpool,
            tc.tile_pool(name="accps", bufs=1, space="PSUM") as accpool,
        ):
            wT_s = cpool.tile([128, 128], F16, tag="wT")
            bias_s = cpool.tile([128, 1], F32, tag="bias")
            Zh_s = cpool.tile([128, 64], F16, tag="Zh")
            Zg_s = cpool.tile([128, 64], F16, tag="Zg")
            pb_s = cpool.tile([128, 1], F32, tag="pb")
            xTa = cpool.tile([128, B_c, 60], F16, tag="xTa")
            xTb = cpool.tile([128, B_c, 60], F16, tag="xTb")
            arena = cpool.tile([128, B_c, ARQ], F16, tag="hparena")
            sc0_s = cpool.tile([128, H0], F32, tag="sc0_s")
            g0_s = cpool.tile([128, H0], F32, tag="g0_s")
            warm0 = cpool.tile([128, 1], F32, tag="warm0")
            warm1 = cpool.tile([128, 1], F32, tag="warm1")

            # Warmup with no data deps: loads the exp table set on ACT
            # (contains relu too, so exactly one ACT_TABLE_LOAD, at t~0).
            nc.vector.memset(warm0[:], 0.0)
            nc.scalar.activation(warm1[:], warm0[:], AF.Exp)

            # first x chunk, then the consts needed earliest
            c0 = _DMA_CHUNKS[0]
            nc.sync.dma_start(xTa[:, 0:c0, :], xTa_d[:, 0:c0, :])
            nc.sync.dma_start(xTb[:, 0:c0, :], xTb_d[:, 0:c0, :])
            nc.sync.dma_start(wT_s[:], wT_d[:])
            nc.sync.dma_start(bias_s[:], bias_d[:])
            nc.sync.dma_start(Zh_s[:], Zh_d[:])
            nc.sync.dma_start(Zg_s[:], Zg_d[:])
            nc.sync.dma_start(pb_s[:], pb_d[:])
            dma_starts = set(np.cumsum((0,) + _DMA_CHUNKS[:-1]).tolist())
            chunk_of = {}
            pos = 0
            for ch in _DMA_CHUNKS:
                chunk_of[pos] = ch
                pos += ch

            def emit_hp(pass_, r0, nr, parity):
                """Merged multiply over this pass's deltas of one parity for
                rows r0:r0+nr into the arena (local col (d-dlo)*40).

                in0 = x[0:40] broadcast over deltas; in1 = shifted window.
                d=20 writes 40 cols, the last 20 junk -> arena 380:400 pad.
                """
                dlo = 1 + 10 * pass_
                deltas = [d for d in range(dlo, dlo + 10) if d % 2 == parity]
                nd = len(deltas)
                d0 = deltas[0]
                col0 = (d0 - dlo) * FLD
                ob = arena[:, r0:r0 + nr, col0:col0 + 1]
                out_ap = _win(ob, [[ARQ, nr], [2 * FLD, nd], [1, FLD]])
                i0b = xTa[:, r0:r0 + nr, 0:1]
                in0 = _win(i0b, [[60, nr], [0, nd], [1, FLD]])
                if d0 % 2 == 0:
                    i1b = xTa[:, r0:r0 + nr, d0:d0 + 1]
                else:
                    i1b = xTb[:, r0:r0 + nr, d0 - 1:d0]
                in1 = _win(i1b, [[60, nr], [2, nd], [1, FLD]])
                nc.vector.tensor_mul(out_ap, in0, in1)

            def emit_evict(awt, relu3, n, W, on_dve):
                aw_v = awt[:].rearrange("a (u q) -> a u q", q=512)[:, 0:n, 0:W]
                rl_v = relu3[:].rearrange("a (u q) -> a u q", q=ARQ)[:, 0:n, 0:W]
                if on_dve:
                    nc.vector.tensor_scalar(
                        rl_v, aw_v, bias_s[:], 0.0, op0=ALU.add, op1=ALU.max)
                else:
                    nc.scalar.activation(rl_v, aw_v, AF.Relu, bias=bias_s[:])

            def emit_scg(rec, sc_t, g_t, W):
                t, rows = rec
                st, sp = (t == 0), (t == 31)
                for purpose in (0, 1):
                    for (k, relu_mv) in rows:
                        j = k % 4
                        if purpose == 0:
                            dst, Z, mov = sc_t, Zh_s, relu_mv
                        else:
                            dst, Z, mov = g_t, Zg_s, arena[:, k, 0:W]
                        nc.tensor.matmul(
                            dst[32 * j:32 * j + 32, 0:W],
                            Z[:, 32 - t:64 - t],
                            mov,
                            start=st, stop=sp,
                            tile_position=(0, 32 * j),
                            skip_group_check=True,
                        )

            sc_t1 = g_t1 = None
            for pass_ in (0, 1):
                W = H0 if pass_ == 0 else H1
                sc_t = accpool.tile([128, 512], F32, tag="sc")
                g_t = accpool.tile([128, 512], F32, tag="g")
                if pass_ == 1:
                    sc_t1, g_t1 = sc_t, g_t

                scg_q = []
                quad_rows = []
                awt = relu3 = None
                grp = 0
                hp_starts = [0, 4] + list(range(8, 128, 8))
                hp_idx = 0
                for k in range(B_c):
                    if pass_ == 0 and k in dma_starts and k > 0:
                        ch = chunk_of[k]
                        nc.sync.dma_start(xTa[:, k:k + ch, :],
                                          xTa_d[:, k:k + ch, :])
                        nc.sync.dma_start(xTb[:, k:k + ch, :],
                                          xTb_d[:, k:k + ch, :])
                    while hp_idx < len(hp_starts) and hp_starts[hp_idx] == k:
                        r0 = hp_starts[hp_idx]
                        r1 = hp_starts[hp_idx + 1] if hp_idx + 1 < len(
                            hp_starts) else 128
                        emit_hp(pass_, r0, r1 - r0, 1)
                        emit_hp(pass_, r0, r1 - r0, 0)
                        hp_idx += 1

                    slot = k % 3
                    if slot == 0:
                        awt = awpool.tile([128, 1536], F32, tag="aw")
                        relu3 = rpool.tile([128, 3 * ARQ], F16, tag="relu3")
                    bi = nc.tensor.matmul(
                        awt[:, 512 * slot:512 * slot + W],
                        wT_s[:],
                        arena[:, k, 0:W],
                        start=True, stop=True,
                    )
                    if k % 4 != 0:
                        bi.ins.ldweights = False
                    if slot == 2 or k == B_c - 1:
                        on_dve = (grp % dve_mod == 2)
                        emit_evict(awt, relu3, slot + 1, W, on_dve)
                        grp += 1
                    quad_rows.append(
                        (k, relu3[:].rearrange("a (u q) -> a u q", q=ARQ)
                         [:, slot, 0:W]))
                    if len(quad_rows) == 4:
                        t = quad_rows[0][0] // 4
                        if len(scg_q) >= 6:
                            emit_scg(scg_q.pop(0), sc_t, g_t, W)
                        scg_q.append((t, quad_rows))
                        quad_rows = []

                while scg_q:
                    emit_scg(scg_q.pop(0), sc_t, g_t, W)

                if pass_ == 0:
                    # free the accumulator banks for pass 1
                    nc.scalar.copy(sc0_s[:], sc_t[:, 0:H0])
                    nc.vector.tensor_copy(g0_s[:], g_t[:, 0:H0])

            # ---- softmax tail ----
            # logits are bounded (|sc| <~ 45) so exp without max subtraction
            # is safe in fp32 and softmax is exactly shift-invariant.
            exp_s = cpool.tile([128, P], F32, tag="exp_s")
            junk = cpool.tile([128, P], F32, tag="junk")
            denom = cpool.tile([128, 1], F32, tag="denom")
            rden = cpool.tile([128, 1], F32, tag="rden")
            numer = cpool.tile([128, 1], F32, tag="numer")
            outc = cpool.tile([128, 1], F32, tag="outc")
            den2 = cpool.tile([128, 2], F32, tag="den2")
            num2 = cpool.tile([128, 2], F32, tag="num2")

            nc.scalar.activation(exp_s[:, 0:H0], sc0_s[:], AF.Exp,
                                 accum_out=den2[:, 0:1])
            nc.scalar.activation(exp_s[:, H0:P], sc_t1[:, 0:H1], AF.Exp,
                                 accum_out=den2[:, 1:2])
            nc.vector.scalar_tensor_tensor(
                junk[:, 0:H0], exp_s[:, 0:H0], 1.0, g0_s[:],
                op0=ALU.mult, op1=ALU.mult, accum_out=num2[:, 0:1])
            nc.vector.scalar_tensor_tensor(
                junk[:, H0:P], exp_s[:, H0:P], 1.0, g_t1[:, 0:H1],
                op0=ALU.mult, op1=ALU.mult, accum_out=num2[:, 1:2])
            nc.vector.tensor_reduce(numer[:], num2[:], axis=AXIS.X, op=ALU.add)
            nc.vector.tensor_reduce(denom[:], den2[:], axis=AXIS.X, op=ALU.add)
            nc.vector.reciprocal(rden[:], denom[:])
            nc.vector.tensor_mul(outc[:], numer[:], rden[:])
            nc.vector.tensor_scalar_add(outc[:], outc[:], pb_s[:])
            nc.sync.dma_start(out_d[:], outc[:])

    nc.compile()
    return nc


def make_nc(B_c=128, dve_mod=5):
    nc = bacc.Bacc("TRN2", target_bir_lowering=False, debug=False)
    build(nc, B_c=B_c, dve_mod=dve_mod)
    return nc


def perm_for(B_c=128, blocks=None):
    """perm[slot] = global b stored at SBUF slot.

    Slot k belongs to quad k//4 (the one-hot position) and column group
    k%4, so it accumulates into output partition 32*(k%4) + k//4.
    """
    k = np.arange(B_c)
    return 32 * (k % 4) + k // 4


def host_prep_consts(attn_w_w, attn_w_b, attn_h_w, attn_h_b, attn_p_w, attn_p_b):
    wT = np.ascontiguousarray(attn_w_w.T).astype(np.float16)
    bias = attn_w_b.reshape(128, 1).astype(np.float32)
    Zh = np.zeros((128, 64), np.float16)
    Zh[:, 32] = attn_h_w[0].astype(np.float16)
    Zg = np.zeros((128, 64), np.float16)
    Zg[:, 32] = attn_p_w[0].astype(np.float16)
    pb = np.full((128, 1), np.float32(attn_p_b[0]), np.float32)
    return {"wT": wT, "bias": bias, "Zh": Zh, "Zg": Zg, "pb": pb}


def host_prep_x(x_slice, blocks=None):
    # [B_c, F, E] -> two pre-shifted fp16 copies [E, B_c(perm), 60]
    xT = x_slice.transpose(2, 0, 1).astype(np.float16)
    xT = xT[:, perm_for(x_slice.shape[0]), :]
    B_c = x_slice.shape[0]
    xa = np.zeros((128, B_c, 60), np.float16)
    xa[:, :, 0:40] = xT
    xa[:, :, 40:60] = xT[:, :, 0:20]
    xb = np.zeros((128, B_c, 60), np.float16)
    xb[:, :, 0:59] = xa[:, :, 1:60]
    return np.ascontiguousarray(xa), np.ascontiguousarray(xb)


_NC_CACHE = {}


def _get_nc():
    if "nc" not in _NC_CACHE:
        _NC_CACHE["nc"] = make_nc(B_c=128)
    return _NC_CACHE["nc"]


def kernel(x, attn_w_w, attn_w_b, attn_h_w, attn_h_b, attn_p_w, attn_p_b,
           _trace=False):
    from concourse.bass_utils import run_bass_kernel_spmd
    x = np.asarray(x, np.float32)
    consts = host_prep_consts(np.asarray(attn_w_w), np.asarray(attn_w_b),
                              np.asarray(attn_h_w), np.asarray(attn_h_b),
                              np.asarray(attn_p_w), np.asarray(attn_p_b))
    in_maps = []
    for c in range(8):
        m = dict(consts)
        m["xTa"], m["xTb"] = host_prep_x(x[128 * c:128 * (c + 1)])
        in_maps.append(m)
    nc = _get_nc()
    res = run_bass_kernel_spmd(nc, in_maps, list(range(8)), trace=_trace)
    out = np.concatenate([res.results[c]["out"][:, 0] for c in range(8)])
    if _trace:
        return out.astype(np.float32), res
    return out.astype(np.float32)


# revision 22
# speedup vs baseline: 1.1530x; 1.0126x over previous
"""Attentional Factorization Machine kernel for 8 Trainium2 NeuronCores.

Data-parallel over batch: 1024 rows -> 128 per core. The 780 field-pair
products per row are processed in TWO PASSES over pair columns (deltas 1-10
= pairs 0:400, deltas 11-20 = pairs 400:780). Per pass: hp products are
built by cyclic-delta enumeration (fp16, DVE 2x, merged strided instruction
per parity with a stride-0 broadcast first operand); the attention MLP mm1
runs on the PE with W stationary (one <=400-column matmul per row, fp32 into
one PSUM bank of a 3-row mega-tile); relu+bias eviction runs on 3-row
batches (PSUM->SBUF) split between the scalar engine and DVE; per-pair
scores and p_w projections accumulate in fp32 via one-hot stationary matmuls
packed across PE column groups (rows processed in quads, one row per column
group). The pass-0 accumulators are copied to SBUF between passes so each
pass needs only 2 persistent PSUM banks, leaving 6 banks for aw tiles.
Softmax + combine happen on-chip in a [128, 780] layout; exp is applied
without max subtraction (logits are bounded, softmax is shift-invariant).
"""
import sys
for _p in ("/opt/trn_rl_repo",):
    if _p not in sys.path:
        sys.path.insert(0, _p)

import numpy as np

import concourse.bass as bass
import concourse.bacc as bacc
import concourse.mybir as mybir
import concourse.tile as tile
from concourse.ap import AP

F32 = mybir.dt.float32
F16 = mybir.dt.float16
AF = mybir.ActivationFunctionType
ALU = mybir.AluOpType
AXIS = mybir.AxisListType

FLD = 40
NDELTA = 20
P = 780
H0 = 400   # pass-0 pair columns (deltas 1..10)
H1 = 380   # pass-1 real pair columns (deltas 11..20, d=20 has 20 junk)
ARQ = 400  # hp arena per-row stride (one pass half, incl. junk pad)

_DMA_CHUNKS = (8, 8, 16, 16, 16, 16, 16, 16, 16)  # row chunks, 8-aligned


def _win(base, dims):
    """Raw AP from base slice with explicit [stride, size] free dims."""
    pdim = list(base.ap[0])
    return AP(base.tensor, base.offset, [pdim] + [list(d) for d in dims])


def build(nc, B_c=128, dve_mod=5):
    assert B_c == 128

    xTa_d = nc.dram_tensor("xTa", [128, B_c, 60], F16, kind="ExternalInput").ap()
    xTb_d = nc.dram_tensor("xTb", [128, B_c, 60], F16, kind="ExternalInput").ap()
    wT_d = nc.dram_tensor("wT", [128, 128], F16, kind="ExternalInput").ap()
    bias_d = nc.dram_tensor("bias", [128, 1], F32, kind="ExternalInput").ap()
    Zh_d = nc.dram_tensor("Zh", [128, 64], F16, kind="ExternalInput").ap()
    Zg_d = nc.dram_tensor("Zg", [128, 64], F16, kind="ExternalInput").ap()
    pb_d = nc.dram_tensor("pb", [128, 1], F32, kind="ExternalInput").ap()
    out_d = nc.dram_tensor("out", [B_c, 1], F32, kind="ExternalOutput").ap()

    with tile.TileContext(nc) as tc:
        with (
            tc.tile_pool(name="const", bufs=1) as cpool,
            tc.tile_pool(name="relu", bufs=12) as rpool,
            tc.tile_pool(name="awps", bufs=2, space="PSUM") as awpool,
            tc.tile_pool(name="accps", bufs=1, space="PSUM") as accpool,
        ):
            wT_s = cpool.tile([128, 128], F16, tag="wT")
            bias_s = cpool.tile([128, 1], F32, tag="bias")
            Zh_s = cpool.tile([128, 64], F16, tag="Zh")
            Zg_s = cpool.tile([128, 64], F16, tag="Zg")
            pb_s = cpool.tile([128, 1], F32, tag="pb")
            xTa = cpool.tile([128, B_c, 60], F16, tag="xTa")
            xTb = cpool.tile([128, B_c, 60], F16, tag="xTb")
            arena = cpool.tile([128, B_c, ARQ], F16, tag="hparena")
            sc0_s = cpool.tile([128, H0], F32, tag="sc0_s")
            g0_s = cpool.tile([128, H0], F32, tag="g0_s")
            warm0 = cpool.tile([128, 1], F32, tag="warm0")
            warm1 = cpool.tile([128, 1], F32, tag="warm1")
            warmw = cpool.tile([128, 64], F16, tag="warmw")

            # Warmups with no data deps: load the exp table set on ACT
            # (contains relu too, so exactly one ACT_TABLE_LOAD, at t~0) and
            # keep the PE busy through the startup DMA window so the HAM
            # clock gate reaches 2.4 GHz before the first real matmul.
            nc.vector.memset(warm0[:], 0.0)
            nc.scalar.activation(warm1[:], warm0[:], AF.Exp)
            nc.vector.memset(warmw[:], 0.0)
            wmt = awpool.tile([128, 1536], F32, tag="aw")
            for wi in range(230):
                bi = nc.tensor.matmul(wmt[0:64, 0:64], warmw[:], warmw[:],
                                      start=True, stop=True)
                if wi > 0:
                    bi.ins.ldweights = False

            # first two x chunks, then the consts needed earliest; remaining
            # chunks are issued one chunk ahead of consumption
            starts = np.cumsum((0,) + _DMA_CHUNKS[:-1]).tolist()
            chunk_of = dict(zip(starts, _DMA_CHUNKS))
            for s in starts[:2]:
                ch = chunk_of[s]
                nc.sync.dma_start(xTa[:, s:s + ch, :], xTa_d[:, s:s + ch, :])
                nc.sync.dma_start(xTb[:, s:s + ch, :], xTb_d[:, s:s + ch, :])
            nc.sync.dma_start(wT_s[:], wT_d[:])
            nc.sync.dma_start(bias_s[:], bias_d[:])
            nc.sync.dma_start(Zh_s[:], Zh_d[:])
            nc.sync.dma_start(Zg_s[:], Zg_d[:])
            nc.sync.dma_start(pb_s[:], pb_d[:])
            prefetch_of = {a: b for a, b in zip(starts[:-1], starts[2:])}

            def emit_hp(pass_, r0, nr, parity):
                """Merged multiply over this pass's deltas of one parity for
                rows r0:r0+nr into the arena (local col (d-dlo)*40).

                in0 = x[0:40] broadcast over deltas; in1 = shifted window.
                d=20 writes 40 cols, the last 20 junk -> arena 380:400 pad.
                """
                dlo = 1 + 10 * pass_
                deltas = [d for d in range(dlo, dlo + 10) if d % 2 == parity]
                nd = len(deltas)
                d0 = deltas[0]
                col0 = (d0 - dlo) * FLD
                ob = arena[:, r0:r0 + nr, col0:col0 + 1]
                out_ap = _win(ob, [[ARQ, nr], [2 * FLD, nd], [1, FLD]])
                i0b = xTa[:, r0:r0 + nr, 0:1]
                in0 = _win(i0b, [[60, nr], [0, nd], [1, FLD]])
                if d0 % 2 == 0:
                    i1b = xTa[:, r0:r0 + nr, d0:d0 + 1]
                else:
                    i1b = xTb[:, r0:r0 + nr, d0 - 1:d0]
                in1 = _win(i1b, [[60, nr], [2, nd], [1, FLD]])
                nc.vector.tensor_mul(out_ap, in0, in1)

            def emit_evict(awt, relu3, n, W, on_dve):
                aw_v = awt[:].rearrange("a (u q) -> a u q", q=512)[:, 0:n, 0:W]
                rl_v = relu3[:].rearrange("a (u q) -> a u q", q=ARQ)[:, 0:n, 0:W]
                if on_dve:
                    nc.vector.tensor_scalar(
                        rl_v, aw_v, bias_s[:], 0.0, op0=ALU.add, op1=ALU.max)
                else:
                    nc.scalar.activation(rl_v, aw_v, AF.Relu, bias=bias_s[:])

            def emit_scg(rec, sc_t, g_t, W):
                t, rows = rec
                st, sp = (t == 0), (t == 31)
                for purpose in (0, 1):
                    for (k, relu_mv) in rows:
                        j = k % 4
                        if purpose == 0:
                            dst, Z, mov = sc_t, Zh_s, relu_mv
                        else:
                            dst, Z, mov = g_t, Zg_s, arena[:, k, 0:W]
                        nc.tensor.matmul(
                            dst[32 * j:32 * j + 32, 0:W],
                            Z[:, 32 - t:64 - t],
                            mov,
                            start=st, stop=sp,
                            tile_position=(0, 32 * j),
                            skip_group_check=True,
                        )

            sc_t1 = g_t1 = None
            for pass_ in (0, 1):
                W = H0 if pass_ == 0 else H1
                sc_t = accpool.tile([128, 512], F32, tag="sc")
                g_t = accpool.tile([128, 512], F32, tag="g")
                if pass_ == 1:
                    sc_t1, g_t1 = sc_t, g_t

                scg_q = []
                quad_rows = []
                awt = relu3 = None
                grp = 0
                if pass_ == 0:
                    hp_starts = [0, 4] + list(range(8, 128, 8))
                else:
                    hp_starts = list(range(24, 128, 8))  # 0:24 pre-built
                hp_idx = 0
                for k in range(B_c):
                    if pass_ == 0 and k in prefetch_of:
                        s = prefetch_of[k]
                        ch = chunk_of[s]
                        nc.sync.dma_start(xTa[:, s:s + ch, :],
                                          xTa_d[:, s:s + ch, :])
                        nc.sync.dma_start(xTb[:, s:s + ch, :],
                                          xTb_d[:, s:s + ch, :])
                    if pass_ == 0 and k == 104:
                        # head of pass-1 hp: arena rows 0:24 are long done
                        # being read; build them while pass-0 finishes
                        for r0, r1 in ((0, 8), (8, 16), (16, 24)):
                            emit_hp(1, r0, r1 - r0, 1)
                            emit_hp(1, r0, r1 - r0, 0)
                    while hp_idx < len(hp_starts) and hp_starts[hp_idx] == k:
                        r0 = hp_starts[hp_idx]
                        r1 = hp_starts[hp_idx + 1] if hp_idx + 1 < len(
                            hp_starts) else 128
                        emit_hp(pass_, r0, r1 - r0, 1)
                        emit_hp(pass_, r0, r1 - r0, 0)
                        hp_idx += 1

                    slot = k % 3
                    if slot == 0:
                        awt = awpool.tile([128, 1536], F32, tag="aw")
                        relu3 = rpool.tile([128, 3 * ARQ], F16, tag="relu3")
                    bi = nc.tensor.matmul(
                        awt[:, 512 * slot:512 * slot + W],
                        wT_s[:],
                        arena[:, k, 0:W],
                        start=True, stop=True,
                    )
                    if k % 4 != 0:
                        bi.ins.ldweights = False
                    if slot == 2 or k == B_c - 1:
                        on_dve = (grp % dve_mod == 2)
                        emit_evict(awt, relu3, slot + 1, W, on_dve)
                        grp += 1
                    quad_rows.append(
                        (k, relu3[:].rearrange("a (u q) -> a u q", q=ARQ)
                         [:, slot, 0:W]))
                    if len(quad_rows) == 4:
                        t = quad_rows[0][0] // 4
                        if len(scg_q) >= 6:
                            emit_scg(scg_q.pop(0), sc_t, g_t, W)
                        scg_q.append((t, quad_rows))
                        quad_rows = []

                while scg_q:
                    emit_scg(scg_q.pop(0), sc_t, g_t, W)

                if pass_ == 0:
                    # free the accumulator banks for pass 1
                    nc.scalar.copy(sc0_s[:], sc_t[:, 0:H0])
                    nc.vector.tensor_copy(g0_s[:], g_t[:, 0:H0])

            # ---- softmax tail ----
            # logits are bounded (|sc| <~ 45) so exp without max subtraction
            # is safe in fp32 and softmax is exactly shift-invariant.
            exp_s = cpool.tile([128, P], F32, tag="exp_s")
            junk = cpool.tile([128, P], F32, tag="junk")
            denom = cpool.tile([128, 1], F32, tag="denom")
            rden = cpool.tile([128, 1], F32, tag="rden")
            numer = cpool.tile([128, 1], F32, tag="numer")
            outc = cpool.tile([128, 1], F32, tag="outc")
            den2 = cpool.tile([128, 2], F32, tag="den2")
            num2 = cpool.tile([128, 2], F32, tag="num2")

            nc.scalar.activation(exp_s[:, 0:H0], sc0_s[:], AF.Exp,
                                 accum_out=den2[:, 0:1])
            nc.scalar.activation(exp_s[:, H0:P], sc_t1[:, 0:H1], AF.Exp,
                                 accum_out=den2[:, 1:2])
            nc.vector.scalar_tensor_tensor(
                junk[:, 0:H0], exp_s[:, 0:H0], 1.0, g0_s[:],
                op0=ALU.mult, op1=ALU.mult, accum_out=num2[:, 0:1])
            nc.vector.scalar_tensor_tensor(
                junk[:, H0:P], exp_s[:, H0:P], 1.0, g_t1[:, 0:H1],
                op0=ALU.mult, op1=ALU.mult, accum_out=num2[:, 1:2])
            nc.vector.tensor_reduce(numer[:], num2[:], axis=AXIS.X, op=ALU.add)
            nc.vector.tensor_reduce(denom[:], den2[:], axis=AXIS.X, op=ALU.add)
            nc.vector.reciprocal(rden[:], denom[:])
            nc.vector.tensor_mul(outc[:], numer[:], rden[:])
            nc.vector.tensor_scalar_add(outc[:], outc[:], pb_s[:])
            nc.sync.dma_start(out_d[:], outc[:])

    nc.compile()
    return nc


def make_nc(B_c=128, dve_mod=5):
    nc = bacc.Bacc("TRN2", target_bir_lowering=False, debug=False)
    build(nc, B_c=B_c, dve_mod=dve_mod)
    return nc


def perm_for(B_c=128, blocks=None):
    """perm[slot] = global b stored at SBUF slot.

    Slot k belongs to quad k//4 (the one-hot position) and column group
    k%4, so it accumulates into output partition 32*(k%4) + k//4.
    """
    k = np.arange(B_c)
    return 32 * (k % 4) + k // 4


def host_prep_consts(attn_w_w, attn_w_b, attn_h_w, attn_h_b, attn_p_w, attn_p_b):
    wT = np.ascontiguousarray(attn_w_w.T).astype(np.float16)
    bias = attn_w_b.reshape(128, 1).astype(np.float32)
    Zh = np.zeros((128, 64), np.float16)
    Zh[:, 32] = attn_h_w[0].astype(np.float16)
    Zg = np.zeros((128, 64), np.float16)
    Zg[:, 32] = attn_p_w[0].astype(np.float16)
    pb = np.full((128, 1), np.float32(attn_p_b[0]), np.float32)
    return {"wT": wT, "bias": bias, "Zh": Zh, "Zg": Zg, "pb": pb}


def host_prep_x(x_slice, blocks=None):
    # [B_c, F, E] -> two pre-shifted fp16 copies [E, B_c(perm), 60]
    xT = x_slice.transpose(2, 0, 1).astype(np.float16)
    xT = xT[:, perm_for(x_slice.shape[0]), :]
    B_c = x_slice.shape[0]
    xa = np.zeros((128, B_c, 60), np.float16)
    xa[:, :, 0:40] = xT
    xa[:, :, 40:60] = xT[:, :, 0:20]
    xb = np.zeros((128, B_c, 60), np.float16)
    xb[:, :, 0:59] = xa[:, :, 1:60]
    return np.ascontiguousarray(xa), np.ascontiguousarray(xb)


_NC_CACHE = {}


def _get_nc():
    if "nc" not in _NC_CACHE:
        _NC_CACHE["nc"] = make_nc(B_c=128)
    return _NC_CACHE["nc"]


def kernel(x, attn_w_w, attn_w_b, attn_h_w, attn_h_b, attn_p_w, attn_p_b,
           _trace=False):
    from concourse.bass_utils import run_bass_kernel_spmd
    x = np.asarray(x, np.float32)
    consts = host_prep_consts(np.asarray(attn_w_w), np.asarray(attn_w_b),
                              np.asarray(attn_h_w), np.asarray(attn_h_b),
                              np.asarray(attn_p_w), np.asarray(attn_p_b))
    in_maps = []
    for c in range(8):
        m = dict(consts)
        m["xTa"], m["xTb"] = host_prep_x(x[128 * c:128 * (c + 1)])
        in_maps.append(m)
    nc = _get_nc()
    res = run_bass_kernel_spmd(nc, in_maps, list(range(8)), trace=_trace)
    out = np.concatenate([res.results[c]["out"][:, 0] for c in range(8)])
    if _trace:
        return out.astype(np.float32), res
    return out.astype(np.float32)
